# revision 9
# baseline (speedup 1.0000x reference)
"""Trainium2 Bass kernel for nn_LocalEnergyCore — v3 (balanced-engine version).

Contract: kernel(**inputs) takes FULL unsharded inputs, returns FULL output
(scalar f32). Internally shards z along batch across 8 NeuronCores.

Structure (per core; sites/indices baked into the program at build time):
  - z staged in DRAM as fp8e4m3 rows [(i, j, k), b] with toroidal halo
    (0/1 values are exact in fp8; halves gather bytes vs bf16). Split into
    two row-halves (i<34 / i>=32) so row indices fit int16 for dma_gather.
  - ctx gathers: first 6 sites via direct HWDGE DMAs on the scalar ring
    (no index-table dependency -> PE starts early; no 128-pad overhead);
    the rest via SWDGE dma_gather chunks of <=7 sites (896 idxs; bigger
    gathers exceed the per-SDMA-engine descriptor ring and hang the ucode
    -- HW-verified). Site rows 72..127 of each gather block use dummy
    index 0.
  - L1: 2 sites run concurrently via tile_position col-tiling (0,0)/(0,64);
    moving operand fp8 ctx, stationary bf16 W1 variant [72, 64].
  - relu(h + b1): split DVE (tensor_scalar add+max, 13 pairs) / ACT
    (activation Relu+bias, 12 pairs) to balance engines.
  - L2: per-pair sliding-window slice of one [128,100] block-diag W2
    buffer; even pairs accumulate into logitsA (PSUM cols 2-3), odd pairs
    into logitsB (cols 0-1) to balance PE column groups.
  - targets are host-pre-gathered (50 rows, O(S*B) bookkeeping; the O(72*S*B)
    window gather stays on device) and DMA'd as a [50, 512] f32 input.
  - compare+count via DVE scalar_tensor_tensor accum -> masked ones-matmul
    partition reduce -> scale. Host sums the 8 per-core partials.
"""

import sys

for _p in ("/opt/trn_rl_repo",):
    if _p not in sys.path:
        sys.path.insert(0, _p)

import numpy as np
import ml_dtypes

B, K, H, W = 4096, 8, 64, 64
S, HID, CTX = 50, 64, 71
N_CORES = 8
B_LOC = B // N_CORES
HP, WP = H + 2, W + 2          # padded (halo) field
ROWS_HALF = 34 * WP * K        # rows per split z tensor (i<34 | i>=32)
EB = B_LOC                     # one (i,j,k) row = 512 fp8 bytes

F8 = ml_dtypes.float8_e4m3fn
BF16 = ml_dtypes.bfloat16

LAST_RESULTS = None  # test harness introspection
LAST_NC = None       # built program, for TimelineSim in test.py

N_WARM = 3           # HAM warmup matmuls
N_DIRECT = 6         # leading sites gathered by direct HWDGE DMA
MAX_GATHER_SITES = 7  # 896 idxs; hard cap (SWDGE ring capacity)


def _row(i_local, j, k):
    return (i_local * WP + j) * K + k


def _host_prep(z, W1, b1, W2, b2, b_idx, i_idx, j_idx):
    b_idx = np.asarray(b_idx).astype(np.int64)
    i_idx = np.asarray(i_idx).astype(np.int64)
    j_idx = np.asarray(j_idx).astype(np.int64)

    # site order: A-region (i0 <= 31, windows in rows 0..33) first, then B
    raw = [(int(b_idx[s]), int(i_idx[s]), int(j_idx[s])) for s in range(S)]
    a_sites = [t for t in raw if t[1] <= 31]
    b_sites = [t for t in raw if t[1] > 31]
    sites = a_sites + b_sites
    n_a = len(a_sites)

    # ctx gather chunks (after the N_DIRECT leading direct-DMA sites)
    chunks = []

    def _split(run_start, run_len, src):
        pos = 0
        while pos < run_len:
            n = min(MAX_GATHER_SITES, run_len - pos)
            chunks.append((src, run_start + pos, n))
            pos += n

    n_direct = min(N_DIRECT, n_a)  # direct sites all come from region A
    _split(n_direct, n_a - n_direct, "A")
    _split(n_a, S - n_a, "B")

    # ---- index table: [128, total_cols] int16 ----
    # chunk: num_idxs = 128*n, idx i = s_loc*128 + c; i -> [i%16, col0+i//16]
    idx_np = np.zeros((128, sum(8 * n for (_, _, n) in chunks)), dtype=np.int16)
    col0 = 0
    chunk_meta = []  # (src, start_site, n, col0)
    for (src, s0, n) in chunks:
        off = 0 if src == "A" else 32
        for s_loc in range(n):
            v, i0, j0 = sites[s0 + s_loc]
            for c in range(128):
                i = s_loc * 128 + c
                if c < 72:
                    di, r = divmod(c, 24)
                    dj, k = divmod(r, K)
                    val = _row(i0 + di - off, j0 + dj, k)
                else:
                    val = 0
                idx_np[i % 16, col0 + i // 16] = val
        chunk_meta.append((src, s0, n, col0))
        col0 += 8 * n
    idx_cols = max(col0, 16)
    if idx_np.shape[1] < idx_cols:
        idx_np = np.pad(idx_np, ((0, 0), (0, idx_cols - idx_np.shape[1])))
    for c in range(1, 8):  # replicate across the 8 gpsimd cores
        idx_np[16 * c:16 * (c + 1), :] = idx_np[0:16, :]

    # ---- W1 variants: [72, K*HID] bf16, ctx order c=(di,dj,k) equals
    # original position-major order (di*24+dj*8+k == (3di+dj)*8+k) ----
    t = np.arange(72)
    w1v = np.zeros((72, K * HID), dtype=np.float32)
    for v in range(K):
        drop = 4 * K + v
        src_rows = t - (t > drop)
        m = W1[np.minimum(src_rows, CTX - 1)]
        m[drop] = 0.0
        w1v[:, v * HID:(v + 1) * HID] = m
    w1v_np = np.ascontiguousarray(w1v).astype(BF16)

    # ---- sliding-window block-diag W2: [128, 100], cols 48/49 hold W2 ----
    w2win = np.zeros((128, 100), dtype=np.float32)
    w2win[0:HID, 48] = W2
    w2win[HID:128, 49] = W2
    w2win_np = w2win.astype(BF16)

    # ---- duplicated b1 [128, 1]; the count mask is applied host-side ----
    bo_np = np.concatenate([b1, b1]).astype(np.float32).reshape(128, 1)
    # ---- z shards: padded [66, 66, K, B_LOC] fp8, split into A/B halves,
    #      plus host-pre-gathered targets [S, B_LOC] f32 ----
    in_maps = []
    zf = z.astype(np.float32)
    for c in range(N_CORES):
        zt = zf[c * B_LOC:(c + 1) * B_LOC]                  # [Bl, K, H, W]
        zt = np.transpose(zt, (2, 3, 1, 0))                 # [H, W, K, Bl]
        zp = np.empty((HP, WP, K, B_LOC), dtype=np.float32)
        zp[1:H + 1, 1:W + 1] = zt
        zp[0, 1:W + 1] = zt[H - 1]
        zp[H + 1, 1:W + 1] = zt[0]
        zp[:, 0] = zp[:, W]
        zp[:, W + 1] = zp[:, 1]
        tgt = np.empty((S, B_LOC), dtype=np.float32)
        for s, (v, i0, j0) in enumerate(sites):
            tgt[s] = zp[1 + i0, 1 + j0, v]
        z8 = zp.astype(F8)
        za = np.ascontiguousarray(z8[0:34]).reshape(ROWS_HALF, EB)
        zb = np.ascontiguousarray(z8[32:66]).reshape(ROWS_HALF, EB)
        in_maps.append({
            "zA": za, "zB": zb, "idx": idx_np, "tgt": tgt,
            "w1v": w1v_np, "w2win": w2win_np, "bo": bo_np,
        })
    return in_maps, sites, chunk_meta, n_direct, idx_cols, -float(np.asarray(b2))


def _build_program(sites, chunk_meta, n_direct, idx_cols, neg_b2):
    import concourse.bacc as bacc
    import concourse.mybir as mybir
    import concourse.tile as tile

    fp32 = mybir.dt.float32
    bf16 = mybir.dt.bfloat16
    f8 = mybir.dt.float8e4
    i16 = mybir.dt.int16
    Alu = mybir.AluOpType
    Act = mybir.ActivationFunctionType

    nc = bacc.Bacc("TRN2", target_bir_lowering=False, debug=False,
                   num_devices=N_CORES)

    zA = nc.dram_tensor("zA", [ROWS_HALF, EB], f8, kind="ExternalInput")
    zB = nc.dram_tensor("zB", [ROWS_HALF, EB], f8, kind="ExternalInput")
    idx_d = nc.dram_tensor("idx", [128, idx_cols], i16, kind="ExternalInput")
    tgt_d = nc.dram_tensor("tgt", [S, EB], fp32, kind="ExternalInput")
    w1v_d = nc.dram_tensor("w1v", [72, K * HID], bf16, kind="ExternalInput")
    w2w_d = nc.dram_tensor("w2win", [128, 100], bf16, kind="ExternalInput")
    bo_d = nc.dram_tensor("bo", [128, 1], fp32, kind="ExternalInput")
    outp = nc.dram_tensor("out", [128, 1], fp32, kind="ExternalOutput")

    zsrc = {"A": zA, "B": zB}
    # row-structured views for the direct window DMAs: [i, (j,k), b]
    zview = {k: v[:, :].rearrange("(i jk) b -> i jk b", jk=WP * K)
             for k, v in zsrc.items()}
    max_chunk = max(n for (_, _, n, _) in chunk_meta) if chunk_meta else 1

    with tile.TileContext(nc) as tc:
        with (
            tc.tile_pool(name="const", bufs=1) as cpool,
            tc.tile_pool(name="ctxd", bufs=max(n_direct, 1)) as dpool,
            tc.tile_pool(name="ctx", bufs=max(len(chunk_meta), 1)) as ctxpool,
            tc.tile_pool(name="hsb", bufs=4) as hpool,
            tc.tile_pool(name="hps", bufs=6, space="PSUM") as pspool,
            tc.tile_pool(name="lp", bufs=1, space="PSUM") as lppool,
        ):
            # --- critical-path-ordered DMA issue on the sync ring: w1v
            # (gates the first L1), then the direct ctx windows, then the
            # index table (gates the Pool gathers), then late-needed consts.
            # SP is otherwise idle; issuing from ACT would steal ~660ns of
            # its sequencer per DMA from the relu work. ---
            w1v_sb = cpool.tile([72, K * HID], bf16)
            nc.sync.dma_start(out=w1v_sb[:, :], in_=w1v_d[:, :])

            site_slot = {}
            for s in range(n_direct):
                v, i0, j0 = sites[s]
                ct = dpool.tile([72, EB], f8, tag="ctxd")
                nc.sync.dma_start(
                    out=ct[:, :],
                    in_=zview["A"][i0:i0 + 3, j0 * K:(j0 + 3) * K, :])
                site_slot[s] = (ct, None)
                if s == 1:  # index table after the first pair's windows
                    idx_sb = cpool.tile([128, idx_cols], i16)
                    nc.sync.dma_start(out=idx_sb[:, :], in_=idx_d[:, :])
            if n_direct < 2:
                idx_sb = cpool.tile([128, idx_cols], i16)
                nc.sync.dma_start(out=idx_sb[:, :], in_=idx_d[:, :])
            w2w_sb = cpool.tile([128, 100], bf16)
            nc.sync.dma_start(out=w2w_sb[:, :], in_=w2w_d[:, :])
            bo_sb = cpool.tile([128, 1], fp32)
            nc.sync.dma_start(out=bo_sb[:, :], in_=bo_d[:, :])
            b1b_sb = bo_sb[:, 0:1]
            t32 = cpool.tile([S, EB], fp32)
            nc.sync.dma_start(out=t32[:, :], in_=tgt_d[:, :])

            # --- HAM warmup: PE busy on junk while DMAs land. Output goes
            # into lpB: rows 0:64 are cleared by its first start=True L2
            # matmul, rows 64:128 are never read. ---
            warm_sb = cpool.tile([72, 512], bf16)
            nc.vector.memset(warm_sb[:, :], 0.0)
            lpB = lppool.tile([128, EB], fp32, tag="lpB")
            for wi in range(N_WARM):
                q = wi % 2
                nc.tensor.matmul(
                    lpB[q * 64:(q + 1) * 64, :],
                    warm_sb[:, 0:64], warm_sb[:, :],
                    start=True, stop=True, tile_position=(0, q * 64),
                    skip_group_check=True)

            # --- ctx gathers (SWDGE) ---
            for (src, s0, n, col0) in chunk_meta:
                ct = ctxpool.tile([128, max_chunk, EB], f8, tag="ctx")
                nc.gpsimd.dma_gather(
                    out_ap=ct[:, 0:n, :],
                    in_ap=zsrc[src][:, :],
                    idxs_ap=idx_sb[:, col0:col0 + 8 * n],
                    num_idxs=128 * n,
                    num_idxs_reg=128 * n,
                    elem_size=EB,
                )
                for s_loc in range(n):
                    site_slot[s0 + s_loc] = (ct, s_loc)

            # --- main pair loop ---
            lpA = lppool.tile([128, EB], fp32, tag="lpA")
            n_pairs = S // 2
            for p in range(n_pairs):
                sa, sb_ = 2 * p, 2 * p + 1
                (cta, la) = site_slot[sa]
                (ctb, lb) = site_slot[sb_]
                va = sites[sa][0]
                vb = sites[sb_][0]
                h_ps = pspool.tile([128, EB], fp32, tag="hps")
                rhs_a = cta[0:72, :] if la is None else cta[0:72, la:la + 1, :]
                rhs_b = ctb[0:72, :] if lb is None else ctb[0:72, lb:lb + 1, :]
                nc.tensor.matmul(
                    h_ps[0:HID, :],
                    w1v_sb[:, va * HID:(va + 1) * HID],
                    rhs_a,
                    start=True, stop=True, tile_position=(0, 0),
                    skip_group_check=True)
                nc.tensor.matmul(
                    h_ps[HID:128, :],
                    w1v_sb[:, vb * HID:(vb + 1) * HID],
                    rhs_b,
                    start=True, stop=True, tile_position=(0, 64),
                    skip_group_check=True)
                h_sb = hpool.tile([128, EB], bf16, tag="hsb")
                if p % 2 == 1:              # 12 on DVE, 13 on ACT
                    nc.vector.tensor_scalar(
                        out=h_sb[:, :], in0=h_ps[:, :],
                        scalar1=b1b_sb, scalar2=0.0,
                        op0=Alu.add, op1=Alu.max)
                else:
                    nc.scalar.activation(
                        out=h_sb[:, :], in_=h_ps[:, :],
                        func=Act.Relu, bias=b1b_sb, scale=1.0)
                if p % 2 == 0:
                    nc.tensor.matmul(
                        lpA[64:64 + S, :],
                        w2w_sb[:, 48 - 2 * p:98 - 2 * p],
                        h_sb[:, :],
                        start=(p == 0), stop=(p == n_pairs - 1),
                        tile_position=(0, 64), skip_group_check=True)
                else:
                    nc.tensor.matmul(
                        lpB[0:S, :],
                        w2w_sb[:, 48 - 2 * p:98 - 2 * p],
                        h_sb[:, :],
                        start=(p == 1), stop=(p == n_pairs - 2),
                        tile_position=(0, 0), skip_group_check=True)

            # --- compare + count ---
            junk = cpool.tile([S, EB], fp32)
            counts = cpool.tile([128, 1], fp32)
            nc.vector.memset(counts[:, :], 0.0)
            nc.vector.scalar_tensor_tensor(
                out=junk[:, :], in0=lpA[64:64 + S, :], scalar=neg_b2,
                in1=t32[:, :],
                op0=Alu.is_gt, op1=Alu.not_equal,
                accum_out=counts[0:S, 0:1])
            nc.vector.scalar_tensor_tensor(
                out=junk[:, :], in0=lpB[0:S, :], scalar=neg_b2,
                in1=t32[:, :],
                op0=Alu.is_gt, op1=Alu.not_equal,
                accum_out=counts[64:64 + S, 0:1])

            nc.sync.dma_start(out=outp[:, :], in_=counts[:, :])

    nc.compile()
    return nc


def kernel(**inputs):
    global LAST_RESULTS, LAST_NC
    from concourse.bass_utils import run_bass_kernel_spmd

    z = np.asarray(inputs["z"], dtype=np.float32)
    in_maps, sites, chunk_meta, n_direct, idx_cols, neg_b2 = _host_prep(
        z, np.asarray(inputs["W1"], dtype=np.float32),
        np.asarray(inputs["b1"], dtype=np.float32),
        np.asarray(inputs["W2"], dtype=np.float32),
        inputs["b2"], inputs["b_idx"], inputs["i_idx"], inputs["j_idx"])

    nc = _build_program(sites, chunk_meta, n_direct, idx_cols, neg_b2)
    LAST_NC = nc

    res = run_bass_kernel_spmd(nc, in_maps, list(range(N_CORES)))
    LAST_RESULTS = res
    mask = np.zeros(128, dtype=np.float64)
    for s in range(S):
        if (s // 2) % 2 == 0:
            mask[s] = 1.0          # lpA counts at rows 0..49
        else:
            mask[64 + s] = 1.0     # lpB counts at rows 64..113
    total = 0.0
    for r in res.results:
        total += float((r["out"][:, 0].astype(np.float64) * mask).sum())
    return np.float32(total / (B * S))
